# revision 7
# baseline (speedup 1.0000x reference)
"""Trainium2 Bass kernel for CrossAttentionFusion.

Reference computation (per sample b):
    q = Wq @ yolo + bq            [32, N]    (N = 64*64 = 4096)
    k = Wk @ vit + bk             [32, N]
    v = Wv @ vit + bv             [256, N]
    A = softmax((q^T k) / sqrt(32), axis=j)         [N, N]
    out = yolo + Wo @ (v @ A^T) + bo                [256, N]

Sharding: data-parallel over batch B=8 across 8 cores; weights replicated.

Device algorithm (per core, one sample):
  - The output projection commutes into V: vo = ((Wo @ Wv) @ vit), giving
    out = yolo + (vo @ A^T) + (Wo @ bv + bo); no O-projection on device.
  - vo^T[j, c] is produced directly by the projection (lhsT = vit chunk);
    nothing is ever transposed on device.
  - attnT[j, i] = k^T q is computed in [j, i] orientation so the softmax
    denominator and the A.V contraction both reduce over the PSUM partition
    axis. QK matmuls have K=32, so 4 j-tiles are packed into the 128-row PE
    array with tile_position row groups (q/k are built 4x-replicated across
    partition groups by col-packed projection matmuls).
  - P = exp(scale * attnT), no max subtraction (|logits| < 1 at this
    problem's scale; exp cannot overflow). One ACT instruction per 4 j-tiles
    ([128, 2048] across 4 PSUM banks) amortizes ACT fixed overhead. P is
    written as fp8e4 (output absmax error validated ~7e-5).
  - U[c, i] = sum_j voT[j, c] P[j, i] runs in fp8e4 with DoubleRow perf mode
    (2 j-tiles per matmul); accumulation is fp32 in PSUM.
  - denom[i] = sum_j P[j, i] via M=1 all-ones matmuls col-packed 4x into one
    PSUM bank (partials at partitions 0/32/64/96), merged + replicated to all
    128 partitions by one masked matmul (sel4), then reciprocal.
  - Epilogue: U * (1/denom) + yolo + (Wo@bv + bo) in fp32.
  - Software pipelining: projections interleave with the first block's
    QK/exp; each block's QK/exp/denom interleaves into the previous block's
    AV stream so PE and ACT stay concurrently busy.
"""

import sys

sys.path.insert(0, "/opt/trn_rl_repo")

import numpy as np
import ml_dtypes

import concourse.bass as bass
import concourse.tile as tile
from concourse import bacc, mybir
from concourse.bass_utils import run_bass_kernel_spmd

BF16 = ml_dtypes.bfloat16
F32 = mybir.dt.float32
BF = mybir.dt.bfloat16
F8 = mybir.dt.float8e4

B, C, H, W = 8, 256, 64, 64
N = H * W            # 4096
CQK = C // 8         # 32
P = 128              # partitions
IB = 512             # i-block (one PSUM bank of fp32)
NIB = N // IB        # 8
JT = N // P          # 32 j-tiles
JG = JT // 4         # 8 groups of 4 j-tiles
CC = C // P          # 2 channel chunks
XCH = 1024           # input DMA column chunk
NXC = N // XCH       # 4
SCALE = 1.0 / float(np.sqrt(np.float32(CQK)))


def build_nc():
    nc = bacc.Bacc("TRN2", target_bir_lowering=False, debug=False)

    x_yolo = nc.dram_tensor("x_yolo", [C, N], F32, kind="ExternalInput")
    x_vit = nc.dram_tensor("x_vit", [C, N], F32, kind="ExternalInput")
    wqt = nc.dram_tensor("wqt", [C, CQK], BF, kind="ExternalInput")
    wkt = nc.dram_tensor("wkt", [C, CQK], BF, kind="ExternalInput")
    wvo = nc.dram_tensor("wvo", [C, C], BF, kind="ExternalInput")  # (Wo@Wv)^T
    bq4 = nc.dram_tensor("bq4", [P, 1], F32, kind="ExternalInput")  # tile(bq,4)
    bk4 = nc.dram_tensor("bk4", [P, 1], F32, kind="ExternalInput")
    bop = nc.dram_tensor("bop", [C, 1], F32, kind="ExternalInput")  # Wo@bv+bo
    sel4 = nc.dram_tensor("sel4", [P, P], BF, kind="ExternalInput")
    out = nc.dram_tensor("out", [C, N], F32, kind="ExternalOutput")

    with tile.TileContext(nc) as tc:
        with (
            tc.tile_pool(name="sg", bufs=1) as sg,
            tc.tile_pool(name="pxv", bufs=4) as pxv,
            tc.tile_pool(name="pp4", bufs=16) as pp4,
            tc.tile_pool(name="pr", bufs=2) as pr,
            tc.tile_pool(name="pot", bufs=4) as pot,
            tc.tile_pool(name="ps_l", bufs=1, space="PSUM") as ps_l,
            tc.tile_pool(name="ps_u", bufs=1, space="PSUM") as ps_u,
            tc.tile_pool(name="ps_den", bufs=1, space="PSUM") as ps_den,
            tc.tile_pool(name="ps_misc", bufs=2, space="PSUM") as ps_misc,
        ):
            # ---- weights / consts ----
            wqt_sb = []
            wkt_sb = []
            wvo_sb = []
            for cc in range(CC):
                csl = slice(cc * P, (cc + 1) * P)
                t = sg.tile([P, CQK], BF, name=f"wqt{cc}")
                nc.sync.dma_start(t[:], wqt[csl, :])
                wqt_sb.append(t)
                t = sg.tile([P, CQK], BF, name=f"wkt{cc}")
                nc.sync.dma_start(t[:], wkt[csl, :])
                wkt_sb.append(t)
                t = sg.tile([P, C], BF, name=f"wvo{cc}")
                nc.sync.dma_start(t[:], wvo[csl, :])
                wvo_sb.append(t)

            bq_sb = sg.tile([P, 1], F32)
            nc.sync.dma_start(bq_sb[:], bq4[:])
            bk_sb = sg.tile([P, 1], F32)
            nc.sync.dma_start(bk_sb[:], bk4[:])
            sel4_sb = sg.tile([P, P], BF)
            nc.sync.dma_start(sel4_sb[:], sel4[:])
            bop_sb = []
            for cc in range(CC):
                t = sg.tile([P, 1], F32, name=f"bop{cc}")
                nc.sync.dma_start(t[:], bop[cc * P : (cc + 1) * P, :])
                bop_sb.append(t)

            ones1 = sg.tile([P, 1], F8)
            nc.vector.memset(ones1[:], 1.0)
            den4_sb = sg.tile([P, IB], BF)
            nc.vector.memset(den4_sb[:], 0.0)

            # ---- chunked input loads + casts (per-chunk tiles so readers
            # start as soon as their chunk lands; vit casts on ACT, yolo on
            # DVE). yolo fp32 stays resident as yb = yolo + bop. ----
            yb = [sg.tile([P, N], F32, name=f"yb{cc}") for cc in range(CC)]
            xy_bf = [
                [sg.tile([P, XCH], BF, name=f"xybf{cc}_{x}") for x in range(NXC)]
                for cc in range(CC)
            ]
            xv_bf = [
                [sg.tile([P, XCH], BF, name=f"xvbf{cc}_{x}") for x in range(NXC)]
                for cc in range(CC)
            ]
            for x in range(NXC):
                xsl = slice(x * XCH, (x + 1) * XCH)
                for cc in range(CC):
                    csl = slice(cc * P, (cc + 1) * P)
                    xvf = pxv.tile([P, XCH], F32, tag="xvf", name="xvf")
                    nc.sync.dma_start(xvf[:], x_vit[csl, xsl])
                    nc.scalar.copy(xv_bf[cc][x][:], xvf[:])
                for cc in range(CC):
                    csl = slice(cc * P, (cc + 1) * P)
                    nc.sync.dma_start(yb[cc][:, xsl], x_yolo[csl, xsl])
                    nc.vector.tensor_copy(xy_bf[cc][x][:], yb[cc][:, xsl])
            for cc in range(CC):
                nc.vector.tensor_scalar_add(
                    out=yb[cc][:], in0=yb[cc][:], scalar1=bop_sb[cc][:]
                )

            # per-ic q/k tiles; per-pair vo tiles (fp8, DoubleRow layout)
            q_t = [sg.tile([P, IB], BF, name=f"q{ic}") for ic in range(NIB)]
            k_t = [sg.tile([P, IB], BF, name=f"k{ic}") for ic in range(NIB)]
            vo_t = [sg.tile([P, 2, C], F8, name=f"vo{t}") for t in range(JT // 2)]

            def emit_qk_proj(dst, wt, bias, src, ic):
                ssl = slice((ic % 2) * IB, (ic % 2 + 1) * IB)
                prj = ps_misc.tile([P, IB], F32, tag="misc", name="prj")
                for g in range(4):
                    for cc in range(CC):
                        nc.tensor.matmul(
                            prj[32 * g : 32 * (g + 1), :],
                            wt[cc][:],
                            src[cc][ic // 2][:, ssl],
                            start=(cc == 0),
                            stop=(cc == CC - 1),
                            tile_position=(0, 32 * g),
                        )
                nc.vector.tensor_scalar_add(out=dst[ic][:], in0=prj[:], scalar1=bias[:])

            def emit_vo_proj(t):
                # two j-tiles (jt = 2t, 2t+1) share one PSUM bank
                vo_ps = ps_misc.tile([P, 2, C], F32, tag="misc", name="vo_ps")
                for u in range(2):
                    jt = 2 * t + u
                    ssl = slice((jt % 8) * P, (jt % 8 + 1) * P)
                    for cc in range(CC):
                        nc.tensor.matmul(
                            vo_ps[:, u, :],
                            xv_bf[cc][jt // 8][:, ssl],
                            wvo_sb[cc][:],
                            start=(cc == 0),
                            stop=(cc == CC - 1),
                        )
                nc.vector.tensor_copy(vo_t[t][:], vo_ps[:])

            # D1 group: QK (4x row-packed) + exp + denom partials (col-packed)
            def emit_d1_group(ib, G, den_ps, p4s):
                l_ps = ps_l.tile([P, 4, IB], F32, tag="l", name="l_ps")
                for g in range(4):
                    jt = 4 * G + g
                    gsl = slice(32 * g, 32 * (g + 1))
                    nc.tensor.matmul(
                        l_ps[:, g, :],
                        k_t[G][gsl, (jt % 4) * P : (jt % 4 + 1) * P],
                        q_t[ib][gsl, :],
                        start=True,
                        stop=True,
                        tile_position=(32 * g, 0),
                    )
                p4 = pp4.tile([P, 4, IB], F8, tag="p4", name="p4")
                nc.scalar.activation(
                    p4[:],
                    l_ps[:],
                    mybir.ActivationFunctionType.Exp,
                    bias=0.0,
                    scale=SCALE,
                )
                p4s.append(p4)
                for g in range(4):
                    nc.tensor.matmul(
                        den_ps[32 * g : 32 * g + 1, :],
                        ones1[:],
                        p4[:, g, :],
                        start=(G == 0),
                        stop=(G == JG - 1),
                        tile_position=(0, 32 * g),
                    )

            def emit_den_finalize(den_ps):
                # 4 partial rows PSUM -> SBUF (walrus forbids partition-strided
                # engine APs, so four row copies; off the PE critical path)
                for g in range(4):
                    nc.vector.tensor_copy(
                        den4_sb[32 * g : 32 * g + 1, :],
                        den_ps[32 * g : 32 * g + 1, :],
                    )

            # ---- Prologue: projections interleaved with D1(ib=0) ----
            den_ps_cur = ps_den.tile([P, IB], F32, tag="den", name="den_ps")
            p4s_cur = []
            for G in range(JG):
                emit_qk_proj(k_t, wkt_sb, bk_sb, xv_bf, G)
                emit_qk_proj(q_t, wqt_sb, bq_sb, xy_bf, G)
                emit_vo_proj(2 * G)
                emit_vo_proj(2 * G + 1)
                emit_d1_group(0, G, den_ps_cur, p4s_cur)
            emit_den_finalize(den_ps_cur)

            # ---- Main loop: D2(ib) with D1(ib+1) interleaved ----
            for ib in range(NIB):
                isl = slice(ib * IB, (ib + 1) * IB)
                p4s = p4s_cur

                if ib + 1 < NIB:
                    den_ps_cur = ps_den.tile([P, IB], F32, tag="den", name="den_ps")
                    p4s_cur = []
                    d1_next = [(ib + 1, G) for G in range(JG)]
                else:
                    d1_next = []

                r_sb = pr.tile([P, IB], F32, tag="r", name="r_sb")
                step = 0
                for cc in range(CC):
                    u_ps = ps_u.tile([P, IB], F32, tag="u", name="u_ps")
                    for G in range(JG):
                        for u in range(2):
                            nc.tensor.matmul(
                                u_ps[:],
                                vo_t[2 * G + u][:, :, cc * P : (cc + 1) * P],
                                p4s[G][:, 2 * u : 2 * u + 2, :],
                                start=(G == 0 and u == 0),
                                stop=(G == JG - 1 and u == 1),
                                perf_mode=mybir.MatmulPerfMode.DoubleRow,
                            )
                        if step == 3:
                            # denom merge: one masked matmul sums rows
                            # {0,32,64,96} and replicates across partitions
                            rep_ps = ps_misc.tile(
                                [P, IB], F32, tag="misc", name="rep_ps"
                            )
                            nc.tensor.matmul(
                                rep_ps[:], sel4_sb[:], den4_sb[:], start=True, stop=True
                            )
                            nc.vector.reciprocal(r_sb[:], rep_ps[:])
                        if step % 2 == 1 and d1_next:
                            nib, nG = d1_next.pop(0)
                            emit_d1_group(nib, nG, den_ps_cur, p4s_cur)
                            if not d1_next:
                                emit_den_finalize(den_ps_cur)
                        step += 1
                    ot = pot.tile([P, IB], F32, tag="ot", name="ot")
                    nc.vector.tensor_mul(ot[:], u_ps[:], r_sb[:])
                    nc.vector.tensor_add(ot[:], ot[:], yb[cc][:, isl])
                    nc.sync.dma_start(out[cc * P : (cc + 1) * P, isl], ot[:])

    nc.compile()
    return nc


_NC_CACHE = {}


def _get_nc():
    if "nc" not in _NC_CACHE:
        _NC_CACHE["nc"] = build_nc()
    return _NC_CACHE["nc"]


def _prep_in_maps(inputs):
    yolo = np.ascontiguousarray(np.asarray(inputs["yolo_features"], np.float32))
    vit = np.ascontiguousarray(np.asarray(inputs["vit_features"], np.float32))
    Wq = np.asarray(inputs["Wq"], np.float32)
    bq = np.asarray(inputs["bq"], np.float32)
    Wk = np.asarray(inputs["Wk"], np.float32)
    bk = np.asarray(inputs["bk"], np.float32)
    Wv = np.asarray(inputs["Wv"], np.float32)
    bv = np.asarray(inputs["bv"], np.float32)
    Wo = np.asarray(inputs["Wo"], np.float32)
    bo = np.asarray(inputs["bo"], np.float32)

    wqt = np.ascontiguousarray(Wq.T).astype(BF16)
    wkt = np.ascontiguousarray(Wk.T).astype(BF16)
    wvo = np.ascontiguousarray((Wo @ Wv).T).astype(BF16)
    bq4 = np.ascontiguousarray(np.tile(bq, 4)[:, None].astype(np.float32))
    bk4 = np.ascontiguousarray(np.tile(bk, 4)[:, None].astype(np.float32))
    bop = np.ascontiguousarray((Wo @ bv + bo)[:, None].astype(np.float32))
    sel4 = np.zeros((P, P), dtype=BF16)
    sel4[[0, 32, 64, 96], :] = 1.0

    in_maps = []
    for b in range(B):
        in_maps.append(
            {
                "x_yolo": yolo[b].reshape(C, N),
                "x_vit": vit[b].reshape(C, N),
                "wqt": wqt,
                "wkt": wkt,
                "wvo": wvo,
                "bq4": bq4,
                "bk4": bk4,
                "bop": bop,
                "sel4": sel4,
            }
        )
    return in_maps


def run(inputs, trace=False):
    nc = _get_nc()
    in_maps = _prep_in_maps(inputs)
    res = run_bass_kernel_spmd(nc, in_maps, list(range(B)), trace=trace)
    out = np.stack([res.results[b]["out"] for b in range(B)], axis=0)
    return out.reshape(B, C, H, W).astype(np.float32), res


def kernel(**inputs):
    out, _ = run(inputs, trace=False)
    return out


# revision 8
# speedup vs baseline: 1.0667x; 1.0667x over previous
"""Trainium2 Bass kernel for CrossAttentionFusion.

Reference computation (per sample b):
    q = Wq @ yolo + bq            [32, N]    (N = 64*64 = 4096)
    k = Wk @ vit + bk             [32, N]
    v = Wv @ vit + bv             [256, N]
    A = softmax((q^T k) / sqrt(32), axis=j)         [N, N]
    out = yolo + Wo @ (v @ A^T) + bo                [256, N]

Sharding: data-parallel over batch B=8 across 8 cores; weights replicated.

Device algorithm (per core, one sample):
  - The output projection commutes into V: vo = ((Wo @ Wv) @ vit), giving
    out = yolo + (vo @ A^T) + (Wo @ bv + bo); no O-projection on device.
  - vo^T[j, c] is produced directly by the projection (lhsT = vit chunk);
    nothing is ever transposed on device.
  - attnT[j, i] = k^T q is computed in [j, i] orientation so the softmax
    denominator and the A.V contraction both reduce over the PSUM partition
    axis. QK matmuls have K=32, so 4 j-tiles are packed into the 128-row PE
    array with tile_position row groups (q/k are built 4x-replicated across
    partition groups by col-packed projection matmuls).
  - P = exp(scale * attnT), no max subtraction (|logits| < 1 at this
    problem's scale; exp cannot overflow). One ACT instruction per 4 j-tiles
    ([128, 2048] across 4 PSUM banks) amortizes ACT fixed overhead. P is
    written as fp8e4 (output absmax error validated ~7e-5).
  - U[c, i] = sum_j voT[j, c] P[j, i] runs in fp8e4 with DoubleRow perf mode
    (2 j-tiles per matmul); accumulation is fp32 in PSUM.
  - denom[i] = sum_j P[j, i] via M=1 all-ones matmuls col-packed 4x into one
    PSUM bank (partials at partitions 0/32/64/96), merged + replicated to all
    128 partitions by one masked matmul (sel4), then reciprocal.
  - Epilogue: U * (1/denom) + yolo + (Wo@bv + bo) in fp32.
  - Software pipelining: projections interleave with the first block's
    QK/exp; each block's QK/exp/denom interleaves into the previous block's
    AV stream so PE and ACT stay concurrently busy.
"""

import sys

sys.path.insert(0, "/opt/trn_rl_repo")

import numpy as np
import ml_dtypes

import concourse.bass as bass
import concourse.tile as tile
from concourse import bacc, mybir
from concourse.bass_utils import run_bass_kernel_spmd

BF16 = ml_dtypes.bfloat16
F32 = mybir.dt.float32
BF = mybir.dt.bfloat16
F8 = mybir.dt.float8e4

B, C, H, W = 8, 256, 64, 64
N = H * W            # 4096
CQK = C // 8         # 32
P = 128              # partitions
IB = 512             # i-block (one PSUM bank of fp32)
NIB = N // IB        # 8
JT = N // P          # 32 j-tiles
JG = JT // 4         # 8 groups of 4 j-tiles
CC = C // P          # 2 channel chunks
XCH = 1024           # input DMA column chunk
NXC = N // XCH       # 4
SCALE = 1.0 / float(np.sqrt(np.float32(CQK)))


def build_nc():
    nc = bacc.Bacc("TRN2", target_bir_lowering=False, debug=False)

    x_yolo = nc.dram_tensor("x_yolo", [C, N], F32, kind="ExternalInput")
    x_vit = nc.dram_tensor("x_vit", [C, N], F32, kind="ExternalInput")
    wqt = nc.dram_tensor("wqt", [C, CQK], BF, kind="ExternalInput")
    wkt = nc.dram_tensor("wkt", [C, CQK], BF, kind="ExternalInput")
    wvo = nc.dram_tensor("wvo", [C, C], BF, kind="ExternalInput")  # (Wo@Wv)^T
    bq4 = nc.dram_tensor("bq4", [P, 1], F32, kind="ExternalInput")  # tile(bq,4)
    bk4 = nc.dram_tensor("bk4", [P, 1], F32, kind="ExternalInput")
    bop = nc.dram_tensor("bop", [C, 1], F32, kind="ExternalInput")  # Wo@bv+bo
    sel4 = nc.dram_tensor("sel4", [P, P], BF, kind="ExternalInput")
    out = nc.dram_tensor("out", [C, N], F32, kind="ExternalOutput")

    with tile.TileContext(nc) as tc:
        with (
            tc.tile_pool(name="sg", bufs=1) as sg,
            tc.tile_pool(name="pxv", bufs=4) as pxv,
            tc.tile_pool(name="pp4", bufs=16) as pp4,
            tc.tile_pool(name="pr", bufs=2) as pr,
            tc.tile_pool(name="pot", bufs=4) as pot,
            tc.tile_pool(name="ps_l", bufs=1, space="PSUM") as ps_l,
            tc.tile_pool(name="ps_u", bufs=1, space="PSUM") as ps_u,
            tc.tile_pool(name="ps_den", bufs=1, space="PSUM") as ps_den,
            tc.tile_pool(name="ps_misc", bufs=2, space="PSUM") as ps_misc,
        ):
            # ---- weights / consts ----
            wqt_sb = []
            wkt_sb = []
            wvo_sb = []
            for cc in range(CC):
                csl = slice(cc * P, (cc + 1) * P)
                t = sg.tile([P, CQK], BF, name=f"wqt{cc}")
                nc.sync.dma_start(t[:], wqt[csl, :])
                wqt_sb.append(t)
                t = sg.tile([P, CQK], BF, name=f"wkt{cc}")
                nc.sync.dma_start(t[:], wkt[csl, :])
                wkt_sb.append(t)
                t = sg.tile([P, C], BF, name=f"wvo{cc}")
                nc.sync.dma_start(t[:], wvo[csl, :])
                wvo_sb.append(t)

            bq_sb = sg.tile([P, 1], F32)
            nc.sync.dma_start(bq_sb[:], bq4[:])
            bk_sb = sg.tile([P, 1], F32)
            nc.sync.dma_start(bk_sb[:], bk4[:])
            sel4_sb = sg.tile([P, P], BF)
            nc.sync.dma_start(sel4_sb[:], sel4[:])
            bop_sb = []
            for cc in range(CC):
                t = sg.tile([P, 1], F32, name=f"bop{cc}")
                nc.sync.dma_start(t[:], bop[cc * P : (cc + 1) * P, :])
                bop_sb.append(t)

            ones1 = sg.tile([P, 1], F8)
            nc.vector.memset(ones1[:], 1.0)
            den4_sb = sg.tile([P, IB], BF)
            nc.vector.memset(den4_sb[:], 0.0)

            # ---- chunked input loads + casts (per-chunk tiles so readers
            # start as soon as their chunk lands; vit casts on ACT, yolo on
            # DVE). yolo fp32 stays resident as yb = yolo + bop. ----
            yb = [sg.tile([P, N], F32, name=f"yb{cc}") for cc in range(CC)]
            xy_bf = [
                [sg.tile([P, XCH], BF, name=f"xybf{cc}_{x}") for x in range(NXC)]
                for cc in range(CC)
            ]
            xv_bf = [
                [sg.tile([P, XCH], BF, name=f"xvbf{cc}_{x}") for x in range(NXC)]
                for cc in range(CC)
            ]
            for x in range(NXC):
                xsl = slice(x * XCH, (x + 1) * XCH)
                for cc in range(CC):
                    csl = slice(cc * P, (cc + 1) * P)
                    xvf = pxv.tile([P, XCH], F32, tag="xvf", name="xvf")
                    nc.sync.dma_start(xvf[:], x_vit[csl, xsl])
                    nc.scalar.copy(xv_bf[cc][x][:], xvf[:])
                for cc in range(CC):
                    csl = slice(cc * P, (cc + 1) * P)
                    nc.sync.dma_start(yb[cc][:, xsl], x_yolo[csl, xsl])
                    nc.vector.tensor_copy(xy_bf[cc][x][:], yb[cc][:, xsl])
            for cc in range(CC):
                nc.vector.tensor_scalar_add(
                    out=yb[cc][:], in0=yb[cc][:], scalar1=bop_sb[cc][:]
                )

            # per-ic q/k tiles; per-pair vo tiles (fp8, DoubleRow layout)
            q_t = [sg.tile([P, IB], BF, name=f"q{ic}") for ic in range(NIB)]
            k_t = [sg.tile([P, IB], BF, name=f"k{ic}") for ic in range(NIB)]
            vo_t = [sg.tile([P, 2, C], F8, name=f"vo{t}") for t in range(JT // 2)]

            def emit_qk_proj(dst, wt, bias, src, ic):
                ssl = slice((ic % 2) * IB, (ic % 2 + 1) * IB)
                prj = ps_misc.tile([P, IB], F32, tag="misc", name="prj")
                for g in range(4):
                    for cc in range(CC):
                        nc.tensor.matmul(
                            prj[32 * g : 32 * (g + 1), :],
                            wt[cc][:],
                            src[cc][ic // 2][:, ssl],
                            start=(cc == 0),
                            stop=(cc == CC - 1),
                            tile_position=(0, 32 * g),
                        )
                nc.vector.tensor_scalar_add(out=dst[ic][:], in0=prj[:], scalar1=bias[:])

            def emit_vo_proj(t):
                # two j-tiles (jt = 2t, 2t+1) share one PSUM bank
                vo_ps = ps_misc.tile([P, 2, C], F32, tag="misc", name="vo_ps")
                for u in range(2):
                    jt = 2 * t + u
                    ssl = slice((jt % 8) * P, (jt % 8 + 1) * P)
                    for cc in range(CC):
                        nc.tensor.matmul(
                            vo_ps[:, u, :],
                            xv_bf[cc][jt // 8][:, ssl],
                            wvo_sb[cc][:],
                            start=(cc == 0),
                            stop=(cc == CC - 1),
                        )
                nc.vector.tensor_copy(vo_t[t][:], vo_ps[:])

            # D1 group: QK (4x row-packed) + exp + denom partials (col-packed)
            def emit_d1_group(ib, G, den_ps, p4s):
                l_ps = ps_l.tile([P, 4, IB], F32, tag="l", name="l_ps")
                for g in range(4):
                    jt = 4 * G + g
                    gsl = slice(32 * g, 32 * (g + 1))
                    nc.tensor.matmul(
                        l_ps[:, g, :],
                        k_t[G][gsl, (jt % 4) * P : (jt % 4 + 1) * P],
                        q_t[ib][gsl, :],
                        start=True,
                        stop=True,
                        tile_position=(32 * g, 0),
                    )
                p4 = pp4.tile([P, 4, IB], F8, tag="p4", name="p4")
                nc.scalar.activation(
                    p4[:],
                    l_ps[:],
                    mybir.ActivationFunctionType.Exp,
                    bias=0.0,
                    scale=SCALE,
                )
                p4s.append(p4)
                for g in range(4):
                    nc.tensor.matmul(
                        den_ps[32 * g : 32 * g + 1, :],
                        ones1[:],
                        p4[:, g, :],
                        start=(G == 0),
                        stop=(G == JG - 1),
                        tile_position=(0, 32 * g),
                    )

            def emit_den_finalize(den_ps):
                # 4 partial rows PSUM -> SBUF (walrus forbids partition-strided
                # engine APs, so four row copies; off the PE critical path)
                for g in range(4):
                    nc.vector.tensor_copy(
                        den4_sb[32 * g : 32 * g + 1, :],
                        den_ps[32 * g : 32 * g + 1, :],
                    )

            # ---- Prologue: projections interleaved with D1(ib=0) ----
            den_ps_cur = ps_den.tile([P, IB], F32, tag="den", name="den_ps")
            p4s_cur = []
            for G in range(JG):
                emit_qk_proj(k_t, wkt_sb, bk_sb, xv_bf, G)
                emit_qk_proj(q_t, wqt_sb, bq_sb, xy_bf, G)
                emit_vo_proj(2 * G)
                emit_vo_proj(2 * G + 1)
                emit_d1_group(0, G, den_ps_cur, p4s_cur)
            emit_den_finalize(den_ps_cur)

            # ---- Main loop: D2(ib) with D1(ib+1) interleaved ----
            for ib in range(NIB):
                isl = slice(ib * IB, (ib + 1) * IB)
                p4s = p4s_cur

                if ib + 1 < NIB:
                    den_ps_cur = ps_den.tile([P, IB], F32, tag="den", name="den_ps")
                    p4s_cur = []
                    d1_next = [(ib + 1, G) for G in range(JG)]
                else:
                    d1_next = []

                r_sb = pr.tile([P, IB], F32, tag="r", name="r_sb")
                step = 0
                for cc in range(CC):
                    u_ps = ps_u.tile([P, IB], F32, tag="u", name="u_ps")
                    for G in range(JG):
                        for u in range(2):
                            nc.tensor.matmul(
                                u_ps[:],
                                vo_t[2 * G + u][:, :, cc * P : (cc + 1) * P],
                                p4s[G][:, 2 * u : 2 * u + 2, :],
                                start=(G == 0 and u == 0),
                                stop=(G == JG - 1 and u == 1),
                                perf_mode=mybir.MatmulPerfMode.DoubleRow,
                            )
                        if step == 3:
                            # denom merge: one masked matmul sums rows
                            # {0,32,64,96} and replicates across partitions
                            rep_ps = ps_misc.tile(
                                [P, IB], F32, tag="misc", name="rep_ps"
                            )
                            nc.tensor.matmul(
                                rep_ps[:], sel4_sb[:], den4_sb[:], start=True, stop=True
                            )
                            nc.vector.reciprocal_approx_fast(r_sb[:], rep_ps[:])
                        if step % 2 == 1 and d1_next:
                            nib, nG = d1_next.pop(0)
                            emit_d1_group(nib, nG, den_ps_cur, p4s_cur)
                            if not d1_next:
                                emit_den_finalize(den_ps_cur)
                        step += 1
                    ot = pot.tile([P, IB], F32, tag="ot", name="ot")
                    nc.vector.tensor_mul(ot[:], u_ps[:], r_sb[:])
                    nc.vector.tensor_add(ot[:], ot[:], yb[cc][:, isl])
                    nc.sync.dma_start(out[cc * P : (cc + 1) * P, isl], ot[:])

    nc.compile()
    return nc


_NC_CACHE = {}


def _get_nc():
    if "nc" not in _NC_CACHE:
        _NC_CACHE["nc"] = build_nc()
    return _NC_CACHE["nc"]


def _prep_in_maps(inputs):
    yolo = np.ascontiguousarray(np.asarray(inputs["yolo_features"], np.float32))
    vit = np.ascontiguousarray(np.asarray(inputs["vit_features"], np.float32))
    Wq = np.asarray(inputs["Wq"], np.float32)
    bq = np.asarray(inputs["bq"], np.float32)
    Wk = np.asarray(inputs["Wk"], np.float32)
    bk = np.asarray(inputs["bk"], np.float32)
    Wv = np.asarray(inputs["Wv"], np.float32)
    bv = np.asarray(inputs["bv"], np.float32)
    Wo = np.asarray(inputs["Wo"], np.float32)
    bo = np.asarray(inputs["bo"], np.float32)

    wqt = np.ascontiguousarray(Wq.T).astype(BF16)
    wkt = np.ascontiguousarray(Wk.T).astype(BF16)
    wvo = np.ascontiguousarray((Wo @ Wv).T).astype(BF16)
    bq4 = np.ascontiguousarray(np.tile(bq, 4)[:, None].astype(np.float32))
    bk4 = np.ascontiguousarray(np.tile(bk, 4)[:, None].astype(np.float32))
    bop = np.ascontiguousarray((Wo @ bv + bo)[:, None].astype(np.float32))
    sel4 = np.zeros((P, P), dtype=BF16)
    sel4[[0, 32, 64, 96], :] = 1.0

    in_maps = []
    for b in range(B):
        in_maps.append(
            {
                "x_yolo": yolo[b].reshape(C, N),
                "x_vit": vit[b].reshape(C, N),
                "wqt": wqt,
                "wkt": wkt,
                "wvo": wvo,
                "bq4": bq4,
                "bk4": bk4,
                "bop": bop,
                "sel4": sel4,
            }
        )
    return in_maps


def run(inputs, trace=False):
    nc = _get_nc()
    in_maps = _prep_in_maps(inputs)
    res = run_bass_kernel_spmd(nc, in_maps, list(range(B)), trace=trace)
    out = np.stack([res.results[b]["out"] for b in range(B)], axis=0)
    return out.reshape(B, C, H, W).astype(np.float32), res


def kernel(**inputs):
    out, _ = run(inputs, trace=False)
    return out
